# revision 12
# baseline (speedup 1.0000x reference)
import sys

if "/opt/trn_rl_repo" not in sys.path:
    sys.path.insert(0, "/opt/trn_rl_repo")

import numpy as np

H, A, O, M, L = 1024, 1024, 8000, 500, 15
M2 = 2 * M  # 1000
NCORES = 8
HS = H // NCORES       # 128  hidden-dim slice per core
AS = A // NCORES       # 128  alignment-dim slice per core
MS = M2 // NCORES      # 125  maxout-dim slice per core
OS = O // NCORES       # 1000 vocab/output-row slice per core
JT = 8                 # j-tiles per core (OS = JT * 125)
JP = OS // JT          # 125 partitions per j-tile
NCH = 4                # i-chunks of the final stage
WCH = O // NCH         # 2000 columns per chunk

# AllGather payload layout (per-core section)
AG_RH = HS * L         # 1920  (r_i * hidden)^T slice, [128,15] row-major
AG_SEC = AG_RH + MS + MS + 1  # 2171 (+1 = val partial)
AG_TOT = AG_SEC * NCORES

_CACHE = {}
LAST_RESULT = None


def _ensure_profile_hook():
    try:
        import antenv.axon_hooks  # noqa: F401
        return
    except ImportError:
        pass
    try:
        import types
        import antenv  # noqa: F401
        from trn_agent_boot.trn_boot import _ntff_profile_via_ctypes
        hook = _ntff_profile_via_ctypes("/opt/axon/libaxon_pjrt.so")
        m = types.ModuleType("antenv.axon_hooks")
        m.get_axon_ntff_profile_hook = lambda: hook
        m.set_axon_ntff_profile_hook = lambda h: None
        sys.modules["antenv.axon_hooks"] = m
    except Exception:
        pass


def _build(va_b):
    import os as _os
    lvl = int(_os.environ.get("K_LVL", 99))
    dbg_nch = int(_os.environ.get("K_NCH", NCH))
    import contextlib
    import concourse.bass as bass  # noqa: F401
    import concourse.tile as tile
    from concourse import bacc, mybir

    dt = mybir.dt
    AF = mybir.ActivationFunctionType
    ALU = mybir.AluOpType
    AX = mybir.AxisListType
    RG = [list(range(NCORES))]

    nc = bacc.Bacc("TRN2", target_bir_lowering=False, debug=False,
                   num_devices=NCORES)

    f32 = dt.float32
    bf16 = dt.bfloat16

    din = {}

    def I(name, shape):
        din[name] = nc.dram_tensor(name, list(shape), f32, kind="ExternalInput")
        return din[name]

    I("y", [O])                 # embed row (replicated)
    I("ht", [H, L])             # hidden^T (replicated)
    I("enct", [2 * H, L])       # encoder_out^T (replicated)
    I("hcol", [H])              # hidden[c_idx]
    I("ecol", [2 * H])          # enc[c_idx] == h_j
    I("evals", [7])
    I("htsl", [HS, L])          # hidden^T slice for this core
    I("wat", [A, AS])           # Wa_w[As].T
    I("uat", [2 * H, AS])       # Ua_w[As].T
    I("abcol", [AS])            # Wa_b + Ua_b slice
    I("vacol", [AS])            # Va_w[0, As]
    I("wrs", [HS, O]); I("wzs", [HS, O]); I("wss", [HS, O])
    I("rbcol", [HS]); I("zbcol", [HS]); I("sbcol", [HS])
    I("urt", [H, HS]); I("uzt", [H, HS]); I("ust", [H, HS])
    I("czt", [2 * H, HS]); I("cst", [2 * H, HS])
    I("uot", [H, MS]); I("cot", [2 * H, MS])
    I("vos", [MS, O])
    I("obcol", [MS])            # Uo_b + Vo_b + Co_b slice
    I("wot", [512, OS])         # Wo_w[Os].T zero-padded 500->512

    out_o = nc.dram_tensor("out_o", [OS, O], f32, kind="ExternalOutput")
    out_si = nc.dram_tensor("out_si", [HS, L], f32, kind="ExternalOutput")
    out_att = nc.dram_tensor("out_att", [8], f32, kind="ExternalOutput")
    out_en = nc.dram_tensor("out_en", [8], f32, kind="ExternalOutput")

    def row1(dram_t):
        return dram_t.ap().rearrange("(o f) -> o f", o=1)

    S = {}  # shared state between stages

    def stage_loads(persist, ph1):
        def col(name, n):
            t = ph1.tile([n, 1], f32, tag=name, name=name)
            nc.sync.dma_start(t[:], din[name].ap().rearrange("(p o) -> p o", o=1))
            return t

        y_row = ph1.tile([1, O], f32, tag="y_row")
        nc.sync.dma_start(y_row[:], row1(din["y"]))
        y_bc = persist.tile([128, O], f32, tag="y_bc")
        nc.gpsimd.partition_broadcast(y_bc[:], y_row[:])
        S["y_bc"] = y_bc

        for nm, kk, w in [("wat", 8, AS), ("uat", 16, AS), ("uot", 8, MS),
                          ("cot", 16, MS), ("urt", 8, HS)]:
            t = ph1.tile([128, kk, w], f32, tag=nm, name=nm)
            nc.sync.dma_start(t[:], din[nm].ap().rearrange("(k p) m -> p k m", p=128))
            S[nm] = t
        hcol_sb = ph1.tile([128, 8], f32, tag="hcol")
        nc.sync.dma_start(hcol_sb[:], din["hcol"].ap().rearrange("(k p) -> p k", p=128))
        ecol_sb = ph1.tile([128, 16], f32, tag="ecol")
        nc.sync.dma_start(ecol_sb[:], din["ecol"].ap().rearrange("(k p) -> p k", p=128))
        ht_sb = ph1.tile([128, 8, L], f32, tag="ht")
        nc.sync.dma_start(ht_sb[:], din["ht"].ap().rearrange("(k p) t -> p k t", p=128))
        enct_sb = ph1.tile([128, 16, L], f32, tag="enct")
        nc.sync.dma_start(enct_sb[:], din["enct"].ap().rearrange("(k p) t -> p k t", p=128))
        htsl_sb = ph1.tile([HS, L], f32, tag="htsl")
        nc.sync.dma_start(htsl_sb[:], din["htsl"].ap())
        evals_sb = ph1.tile([1, 7], f32, tag="evals")
        nc.sync.dma_start(evals_sb[:], row1(din["evals"]))
        S.update(hcol=hcol_sb, ecol=ecol_sb, ht_sb=ht_sb, enct_sb=enct_sb,
                 htsl=htsl_sb, evals=evals_sb)
        for nm, n in [("abcol", AS), ("vacol", AS), ("rbcol", HS), ("zbcol", HS),
                      ("sbcol", HS), ("obcol", MS)]:
            S[nm] = col(nm, n)
        ones_sb = persist.tile([128, 1], f32, tag="ones")
        nc.vector.memset(ones_sb[:], 1.0)
        S["ones"] = ones_sb

    def mv(psg, ph1, tag, lhsT_sb, rhs_sb, nk, m, ncol):
        ps = psg.tile([128, L], f32, tag="ps", name=f"ps_{tag}")
        for k in range(nk):
            rhs = rhs_sb[:, k, :] if ncol > 1 else rhs_sb[:, k:k + 1]
            nc.tensor.matmul(ps[:m, 0:ncol], lhsT_sb[:, k, :], rhs,
                             start=(k == 0), stop=(k == nk - 1))
        sb = ph1.tile([m, ncol], f32, tag=tag, name=tag)
        nc.scalar.copy(sb[:], ps[:m, 0:ncol])
        return sb

    def stage_val(psg, ph1):
        import os as _os2
        sub = int(_os2.environ.get("K_VAL_SUB", 9))
        att_ps = psg.tile([128, L], f32, tag="ps", name="att_ps")
        n_mm = 8 if sub < 2 else 24
        for k in range(8):
            nc.tensor.matmul(att_ps[:, 0:1], S["wat"][:, k, :], S["hcol"][:, k:k + 1],
                             start=(k == 0), stop=(sub < 2 and k == 7))
        if sub >= 2:
            for k in range(16):
                nc.tensor.matmul(att_ps[:, 0:1], S["uat"][:, k, :], S["ecol"][:, k:k + 1],
                                 start=False, stop=(k == 15))
        attT = ph1.tile([128, 1], f32, tag="attT")
        if sub >= 3:
            nc.scalar.activation(attT[:], att_ps[:, 0:1], AF.Tanh, bias=S["abcol"][:])
        else:
            nc.scalar.copy(attT[:], att_ps[:, 0:1])
        valp_sb = ph1.tile([1, 1], f32, tag="valp")
        if sub >= 4:
            val_ps = psg.tile([128, L], f32, tag="ps", name="val_ps")
            nc.tensor.matmul(val_ps[0:1, 0:1], attT[:], S["vacol"][:], start=True,
                             stop=True)
            nc.scalar.copy(valp_sb[:], val_ps[0:1, 0:1])
        else:
            nc.scalar.copy(valp_sb[:], attT[0:1, :])
            nc.sync.dma_start(out_si.ap()[0:1, 0:1], valp_sb[:])
        S["valp"] = valp_sb

    def stage_ttr(ph1, wstream):
        junk = ph1.tile([128, WCH], f32, tag="junk")
        for name, p_ in [("wrs", HS), ("wzs", HS), ("wss", HS), ("vos", MS)]:
            pacc = ph1.tile([128, NCH], f32, tag=f"pacc_{name}", name=f"pacc_{name}")
            for c in range(NCH):
                wtile = wstream.tile([128, WCH], f32, tag="wstream", name="wtile")
                nc.sync.dma_start(wtile[:p_, :], din[name].ap()[:, c * WCH:(c + 1) * WCH])
                nc.vector.tensor_tensor(
                    junk[:p_, :], wtile[:p_, :], S["y_bc"][:p_, c * WCH:(c + 1) * WCH],
                    ALU.mult)
                nc.vector.reduce_sum(pacc[:p_, c:c + 1], junk[:p_, :], AX.X)
            acc = ph1.tile([128, 1], f32, tag=f"acc_{name}", name=f"acc_{name}")
            nc.vector.reduce_sum(acc[:p_, :], pacc[:p_, :], AX.X)
            S[f"acc_{name}"] = acc

    def stage_pre_ag(psg, ph1):
        urh_sb = mv(psg, ph1, "urh", S["urt"], S["ht_sb"], 8, HS, L)
        uo_sb = mv(psg, ph1, "uo", S["uot"], S["hcol"], 8, MS, 1)
        co_sb = mv(psg, ph1, "co", S["cot"], S["ecol"], 16, MS, 1)
        rbias = ph1.tile([128, 1], f32, tag="rbias")
        nc.vector.tensor_tensor(rbias[:HS, :], S["acc_wrs"][:HS, :], S["rbcol"][:], ALU.add)
        r_iT = ph1.tile([HS, L], f32, tag="riT")
        nc.scalar.activation(r_iT[:], urh_sb[:], AF.Sigmoid, bias=rbias[:HS, :])
        rhT = ph1.tile([HS, L], f32, tag="rhT")
        nc.vector.tensor_tensor(rhT[:], r_iT[:], S["htsl"][:], ALU.mult)
        tuv_sb = ph1.tile([MS, 1], f32, tag="tuv")
        nc.vector.scalar_tensor_tensor(tuv_sb[:], uo_sb[:], S["obcol"][:],
                                       S["acc_vos"][:MS, :], ALU.add, ALU.add)
        S.update(rhT=rhT, tuv=tuv_sb, co_sb=co_sb)

    def stage_ag(dram):
        ag_in = dram.tile([AG_SEC], f32, tag="ag_in")
        ag_out = dram.tile([AG_TOT], f32, tag="ag_out")
        nc.sync.dma_start(ag_in[0:AG_RH].rearrange("(p t) -> p t", t=L), S["rhT"][:])
        nc.sync.dma_start(ag_in[AG_RH:AG_RH + MS].rearrange("(p o) -> p o", o=1),
                          S["tuv"][:])
        nc.sync.dma_start(ag_in[AG_RH + MS:AG_RH + 2 * MS].rearrange("(p o) -> p o", o=1),
                          S["co_sb"][:])
        nc.sync.dma_start(ag_in[AG_SEC - 1:AG_SEC].rearrange("(o f) -> o f", o=1),
                          S["valp"][:])
        nc.gpsimd.collective_compute(
            "AllGather", ALU.bypass, replica_groups=RG,
            ins=[ag_in.opt()], outs=[ag_out.opt()])
        S["ag2d"] = ag_out[:].rearrange("(c s) -> c s", s=AG_SEC)

    def stage_att(ph1):
        ag2d = S["ag2d"]
        vals_sb = ph1.tile([1, NCORES], f32, tag="vals")
        with nc.allow_non_contiguous_dma(reason="8 strided scalars"):
            nc.sync.dma_start(vals_sb[:], ag2d[:, AG_SEC - 1:AG_SEC])
        vraw = ph1.tile([1, 1], f32, tag="vraw")
        nc.vector.reduce_sum(vraw[:], vals_sb[:], AX.X)
        vab_sb = ph1.tile([1, 1], f32, tag="vab")
        nc.vector.memset(vab_sb[:], float(va_b))
        val_sb = ph1.tile([1, 1], f32, tag="val_sb")
        nc.scalar.activation(val_sb[:], vraw[:], AF.Identity, bias=vab_sb[:])
        en_sb = ph1.tile([1, 8], f32, tag="en")
        nc.vector.tensor_copy(en_sb[:, 0:7], S["evals"][:])
        nc.scalar.copy(en_sb[:, 7:8], val_sb[:])
        nc.sync.dma_start(row1(out_en), en_sb[:])
        mx = ph1.tile([1, 1], f32, tag="mx")
        nc.vector.reduce_max(mx[:], en_sb[:], AX.X)
        nmx = ph1.tile([1, 1], f32, tag="nmx")
        nc.scalar.mul(nmx[:], mx[:], -1.0)
        ex_sb = ph1.tile([1, 8], f32, tag="ex")
        nc.scalar.activation(ex_sb[:], en_sb[:], AF.Exp, bias=nmx[:])
        sm_sb = ph1.tile([1, 1], f32, tag="sm")
        nc.vector.reduce_sum(sm_sb[:], ex_sb[:], AX.X)
        rs_sb = ph1.tile([1, 1], f32, tag="rs")
        nc.vector.reciprocal(rs_sb[:], sm_sb[:])
        att_sb = ph1.tile([1, 8], f32, tag="att_sb")
        nc.vector.tensor_scalar_mul(att_sb[:], ex_sb[:], rs_sb[:])
        nc.sync.dma_start(row1(out_att), att_sb[:])
        a_sb = ph1.tile([1, 1], f32, tag="a_sb")
        nc.vector.tensor_copy(a_sb[:], att_sb[:, 7:8])
        a_col = ph1.tile([128, 1], f32, tag="a_col")
        nc.gpsimd.partition_broadcast(a_col[:], a_sb[:])
        S.update(a_sb=a_sb, a_col=a_col)

    def stage_gru(psg, ph1):
        ag2d = S["ag2d"]
        for nm in ["uzt", "ust", "czt", "cst"]:
            kk = 8 if nm in ("uzt", "ust") else 16
            t = ph1.tile([128, kk, HS], f32, tag=nm, name=nm)
            nc.sync.dma_start(t[:], din[nm].ap().rearrange("(k p) m -> p k m", p=128))
            S[nm] = t
        uzh_sb = mv(psg, ph1, "uzh", S["uzt"], S["ht_sb"], 8, HS, L)
        cz_sb = mv(psg, ph1, "cz", S["czt"], S["enct_sb"], 16, HS, L)
        cs_sb = mv(psg, ph1, "cs", S["cst"], S["enct_sb"], 16, HS, L)
        rhk_sb = ph1.tile([128, 8, L], f32, tag="rhk")
        for k in range(NCORES):
            nc.sync.dma_start(rhk_sb[:, k, :],
                              ag2d[k, 0:AG_RH].rearrange("(p t) -> p t", t=L))
        usrh_sb = mv(psg, ph1, "usrh", S["ust"], rhk_sb, 8, HS, L)

        zbias = ph1.tile([128, 1], f32, tag="zbias")
        nc.vector.tensor_tensor(zbias[:HS, :], S["acc_wzs"][:HS, :], S["zbcol"][:], ALU.add)
        tmpz = ph1.tile([HS, L], f32, tag="tmpz")
        nc.vector.scalar_tensor_tensor(tmpz[:], cz_sb[:], S["a_col"][:HS, :], uzh_sb[:],
                                       ALU.mult, ALU.add)
        z_iT = ph1.tile([HS, L], f32, tag="ziT")
        nc.scalar.activation(z_iT[:], tmpz[:], AF.Sigmoid, bias=zbias[:HS, :])

        sbias = ph1.tile([128, 1], f32, tag="sbias")
        nc.vector.tensor_tensor(sbias[:HS, :], S["acc_wss"][:HS, :], S["sbcol"][:], ALU.add)
        tmps = ph1.tile([HS, L], f32, tag="tmps")
        nc.vector.scalar_tensor_tensor(tmps[:], cs_sb[:], S["a_col"][:HS, :], usrh_sb[:],
                                       ALU.mult, ALU.add)
        s_tT = ph1.tile([HS, L], f32, tag="stT")
        nc.scalar.activation(s_tT[:], tmps[:], AF.Tanh, bias=sbias[:HS, :])

        dsl = ph1.tile([HS, L], f32, tag="dsl")
        nc.vector.tensor_tensor(dsl[:], s_tT[:], S["htsl"][:], ALU.subtract)
        dz = ph1.tile([HS, L], f32, tag="dz")
        nc.vector.tensor_tensor(dz[:], dsl[:], z_iT[:], ALU.mult)
        si_sb = ph1.tile([HS, L], f32, tag="si")
        nc.vector.tensor_tensor(si_sb[:], dz[:], S["htsl"][:], ALU.add)
        nc.sync.dma_start(out_si.ap(), si_sb[:])

    def stage_wt(psg, ph1, dram, persist):
        ag2d = S["ag2d"]
        tuv_row = ph1.tile([1, M2], f32, tag="tuv_row")
        nc.sync.dma_start(tuv_row[:].rearrange("o (c m) -> o c m", m=MS),
                          ag2d[:, AG_RH:AG_RH + MS])
        co_row = ph1.tile([1, M2], f32, tag="co_row")
        nc.sync.dma_start(co_row[:].rearrange("o (c m) -> o c m", m=MS),
                          ag2d[:, AG_RH + MS:AG_RH + 2 * MS])
        tbuf = ph1.tile([1, M2 + 2], f32, tag="tbuf")
        nc.vector.scalar_tensor_tensor(tbuf[:, 1:M2 + 1], co_row[:], S["a_sb"][:],
                                       tuv_row[:], ALU.mult, ALU.add)
        nc.scalar.copy(tbuf[:, 0:1], tbuf[:, M2:M2 + 1])
        ti_sb = ph1.tile([1, M], f32, tag="ti")
        nc.vector.tensor_tensor(ti_sb[:], tbuf[:, 0:M2:2], tbuf[:, 1:M2 + 1:2], ALU.max)
        t_dram = dram.tile([512], f32, tag="t_dram")
        zpad = ph1.tile([1, 12], f32, tag="zpad")
        nc.vector.memset(zpad[:], 0.0)
        nc.sync.dma_start(t_dram[M:512].rearrange("(o f) -> o f", o=1), zpad[:])
        nc.sync.dma_start(t_dram[0:M].rearrange("(o f) -> o f", o=1), ti_sb[:])
        ticol_sb = ph1.tile([128, 4], f32, tag="ticol")
        nc.sync.dma_start(ticol_sb[:], t_dram[:].rearrange("(k p) -> p k", p=128))

        wo_sb = ph1.tile([128, 4, OS], f32, tag="wo")
        nc.sync.dma_start(wo_sb[:], din["wot"].ap().rearrange("(k p) m -> p k m", p=128))
        wtc_sb = persist.tile([128, JT], f32, tag="wtc")
        for jt in range(JT):
            wt_ps = psg.tile([128, L], f32, tag="ps", name=f"wtps{jt}")
            for k in range(4):
                nc.tensor.matmul(wt_ps[:JP, 0:1], wo_sb[:, k, jt * JP:(jt + 1) * JP],
                                 ticol_sb[:, k:k + 1], start=(k == 0), stop=(k == 3))
            nc.scalar.copy(wtc_sb[:JP, jt:jt + 1], wt_ps[:JP, 0:1])
        S["wtc"] = wtc_sb

    def stage_final(tc, ctx, persist, dramc):
        zpool = ctx.enter_context(tc.tile_pool(name="zpool", bufs=12))
        e2pool = ctx.enter_context(tc.tile_pool(name="e2pool", bufs=2))
        opool = ctx.enter_context(tc.tile_pool(name="opool", bufs=2))
        lsepool = ctx.enter_context(tc.tile_pool(name="lsepool", bufs=2))
        cspool = ctx.enter_context(tc.tile_pool(name="cspool", bufs=1, space="PSUM"))
        rowpool = ctx.enter_context(tc.tile_pool(name="rowpool", bufs=2))
        y_bc, wtc_sb, ones_sb = S["y_bc"], S["wtc"], S["ones"]

        QW = [512, 512, 512, WCH - 3 * 512]
        for ch in range(dbg_nch):
            c0 = ch * WCH
            zts = []
            csps = [cspool.tile([1, QW[q]], f32, tag=f"cs{q}", name=f"cs{q}_{ch}")
                    for q in range(4)]
            for jt in range(JT):
                zt = zpool.tile([128, WCH], bf16, tag="z", name=f"z_{ch}_{jt}")
                nc.scalar.activation(zt[:JP, :], y_bc[:JP, c0:c0 + WCH], AF.Exp,
                                     scale=wtc_sb[:JP, jt:jt + 1])
                zts.append(zt)
                e2 = e2pool.tile([128, WCH], f32, tag="e2", name=f"e2_{ch}_{jt}")
                nc.scalar.activation(e2[:JP, :], zt[:JP, :], AF.Exp)
                qo = 0
                for q in range(4):
                    nc.tensor.matmul(csps[q][:], ones_sb[:JP, :],
                                     e2[:JP, qo:qo + QW[q]],
                                     start=(jt == 0), stop=(jt == JT - 1))
                    qo += QW[q]
            cs_row = rowpool.tile([1, WCH], f32, tag="cs_row", name=f"csr_{ch}")
            qo = 0
            for q in range(4):
                nc.scalar.copy(cs_row[:, qo:qo + QW[q]], csps[q][:])
                qo += QW[q]
            arc_in = dramc.tile([WCH], f32, tag="arc_in", name=f"arci_{ch}")
            arc_out = dramc.tile([WCH], f32, tag="arc_out", name=f"arco_{ch}")
            nc.sync.dma_start(arc_in[:].rearrange("(o f) -> o f", o=1), cs_row[:])
            nc.gpsimd.collective_compute(
                "AllReduce", ALU.add, replica_groups=RG,
                ins=[arc_in.opt()], outs=[arc_out.opt()])
            lse_row = rowpool.tile([1, WCH], f32, tag="lse_row", name=f"lser_{ch}")
            nc.sync.dma_start(lse_row[:], arc_out[:].rearrange("(o f) -> o f", o=1))
            nc.scalar.activation(lse_row[:], lse_row[:], AF.Ln)
            lse_bc = lsepool.tile([128, WCH], f32, tag="lse_bc", name=f"lseb_{ch}")
            nc.gpsimd.partition_broadcast(lse_bc[:], lse_row[:])
            for jt in range(JT):
                ot = opool.tile([128, WCH], f32, tag="ot", name=f"ot_{ch}_{jt}")
                nc.vector.tensor_tensor(ot[:JP, :], zts[jt][:JP, :],
                                        lse_bc[:JP, :], ALU.subtract)
                nc.sync.dma_start(out_o.ap()[jt * JP:(jt + 1) * JP, c0:c0 + WCH],
                                  ot[:JP, :])

    with tile.TileContext(nc) as tc:
        with contextlib.ExitStack() as ctx:
            persist = ctx.enter_context(tc.tile_pool(name="persist", bufs=1))
            dram = ctx.enter_context(tc.tile_pool(name="dram", bufs=1, space="DRAM"))
            dramc = ctx.enter_context(tc.tile_pool(name="dramc", bufs=4, space="DRAM"))
            ph1ctx = contextlib.ExitStack()
            ph1 = ph1ctx.enter_context(tc.tile_pool(name="ph1", bufs=1))
            wstream = ph1ctx.enter_context(tc.tile_pool(name="wstream", bufs=3))

            stage_loads(persist, ph1)
            if _os.environ.get("K_CONSUME"):
                names = _os.environ["K_CONSUME"].split(",")
                cj = ph1.tile([128, 1], f32, tag="cj")
                for i, nm in enumerate(names):
                    t = S[nm]
                    ap = t[:]
                    if len(ap.shape) == 3:
                        ap = ap.rearrange("p a b -> p (a b)")
                    nc.vector.reduce_sum(cj[:ap.shape[0], :], ap, AX.X,
                                         op=ALU.add)
                    nc.sync.dma_start(out_si.ap()[0:ap.shape[0], i:i + 1],
                                      cj[:ap.shape[0], :])
            with tc.tile_pool(name="psg", bufs=3, space="PSUM") as psg:
                if lvl >= 2:
                    stage_val(psg, ph1)
                if lvl >= 3:
                    stage_ttr(ph1, wstream)
                if lvl >= 4:
                    stage_pre_ag(psg, ph1)
                if lvl >= 5:
                    stage_ag(dram)
                if lvl >= 6:
                    stage_att(ph1)
                if lvl >= 7:
                    stage_gru(psg, ph1)
                if lvl >= 8:
                    stage_wt(psg, ph1, dram, persist)
            ph1ctx.close()
            if lvl >= 9:
                stage_final(tc, ctx, persist, dramc)

    nc.compile()
    return nc


def kernel(input_tok, hidden, h_fwd, h_bwd, e_vals, params, c_idx):
    global LAST_RESULT
    _ensure_profile_hook()
    from concourse.bass_utils import run_bass_kernel_spmd

    p = {k: np.asarray(v, np.float32) for k, v in params.items()}
    tok = int(np.asarray(input_tok).reshape(-1)[0])
    ci = int(np.asarray(c_idx))
    hidden = np.asarray(hidden, np.float32)
    h_fwd = np.asarray(h_fwd, np.float32)
    h_bwd = np.asarray(h_bwd, np.float32)
    e_vals = np.asarray(e_vals, np.float32)

    va_b = float(p["Va_b"][0])
    key = round(va_b, 10)
    if key not in _CACHE:
        _CACHE[key] = _build(va_b)
    nc = _CACHE[key]

    y = np.ascontiguousarray(p["embed"][tok])                   # [O]
    hid2 = hidden[:, 0, :]                                      # [L, H]
    enc = np.concatenate([h_fwd[:, 0, :], h_bwd[:, 0, :]], -1)  # [L, 2H]
    ht = np.ascontiguousarray(hid2.T)                           # [H, L]
    enct = np.ascontiguousarray(enc.T)                          # [2H, L]
    hcol = np.ascontiguousarray(hid2[ci])                       # [H]
    ecol = np.ascontiguousarray(enc[ci])                        # [2H]

    rep = {"y": y, "ht": ht, "enct": enct, "hcol": hcol, "ecol": ecol,
           "evals": e_vals}

    in_maps = []
    for c in range(NCORES):
        hs = slice(HS * c, HS * (c + 1))
        ms = slice(MS * c, MS * (c + 1))
        os_ = slice(OS * c, OS * (c + 1))
        wot = np.zeros((512, OS), np.float32)
        wot[:M] = p["Wo_w"][os_].T
        m = dict(rep)
        m.update({
            "htsl": np.ascontiguousarray(ht[hs]),
            "wat": np.ascontiguousarray(p["Wa_w"][hs].T),
            "uat": np.ascontiguousarray(p["Ua_w"][hs].T),
            "abcol": np.ascontiguousarray(p["Wa_b"][hs] + p["Ua_b"][hs]),
            "vacol": np.ascontiguousarray(p["Va_w"][0, hs]),
            "wrs": np.ascontiguousarray(p["Wr_w"][hs]),
            "wzs": np.ascontiguousarray(p["Wz_w"][hs]),
            "wss": np.ascontiguousarray(p["Ws_w"][hs]),
            "rbcol": np.ascontiguousarray(p["Wr_b"][hs] + p["Ur_b"][hs]),
            "zbcol": np.ascontiguousarray(p["Wz_b"][hs] + p["Uz_b"][hs] + p["Cz_b"][hs]),
            "sbcol": np.ascontiguousarray(p["Ws_b"][hs] + p["Us_b"][hs] + p["Cs_b"][hs]),
            "urt": np.ascontiguousarray(p["Ur_w"][hs].T),
            "uzt": np.ascontiguousarray(p["Uz_w"][hs].T),
            "ust": np.ascontiguousarray(p["Us_w"][hs].T),
            "czt": np.ascontiguousarray(p["Cz_w"][hs].T),
            "cst": np.ascontiguousarray(p["Cs_w"][hs].T),
            "uot": np.ascontiguousarray(p["Uo_w"][ms].T),
            "cot": np.ascontiguousarray(p["Co_w"][ms].T),
            "vos": np.ascontiguousarray(p["Vo_w"][ms]),
            "obcol": np.ascontiguousarray(p["Uo_b"][ms] + p["Vo_b"][ms] + p["Co_b"][ms]),
            "wot": wot,
        })
        in_maps.append(m)

    res = run_bass_kernel_spmd(nc, in_maps, list(range(NCORES)))
    LAST_RESULT = res

    out = np.concatenate([res.results[c]["out_o"] for c in range(NCORES)], 0)
    out = out[None]                                             # [1, O, O]
    siT = np.concatenate([res.results[c]["out_si"] for c in range(NCORES)], 0)
    s_i = np.ascontiguousarray(siT.T)[:, None, :]               # [L, 1, H]
    attention = res.results[0]["out_att"]
    e_new = res.results[0]["out_en"]
    return (out, s_i, attention, e_new)
